# revision 2
# baseline (speedup 1.0000x reference)
"""Row-wise L2-norm clip + noise add (DP-SGD style), data-parallel over 8 cores.

out[i] = x[i] * (1 / max(||x[i]||_2, 1)) + noise[i],  x: [524288, 128] f32

Sharding: pure DP — rows split evenly across 8 NeuronCores, zero comms.
Per-core layout: blocks of 1024 rows; each SBUF tile packs 8 consecutive
rows per partition ([128 part, 8*128 f32] contiguous per-partition DMA).
ACT engine computes per-row sum-of-squares (Square activation + accum_out),
DVE applies the fused (x * scale) + noise via scalar_tensor_tensor.
"""

import sys

import numpy as np

if "/opt/trn_rl_repo" not in sys.path:
    sys.path.insert(0, "/opt/trn_rl_repo")

N, D = 524288, 128
NCORES = 8
N_LOC = N // NCORES            # 65536 rows per core
RPP = 8                        # rows packed per partition per block
BLOCK_ROWS = 128 * RPP         # 1024
N_BLOCKS = N_LOC // BLOCK_ROWS # 64
FREE = RPP * D                 # 1024 f32 per partition per tile

_NC_CACHE = None


def _build():
    global _NC_CACHE
    if _NC_CACHE is not None:
        return _NC_CACHE
    import concourse.bacc as bacc
    import concourse.mybir as mybir
    import concourse.tile as tile

    f32 = mybir.dt.float32
    nc = bacc.Bacc("TRN2", target_bir_lowering=False, debug=False)
    x_d = nc.dram_tensor("x", [N_LOC, D], f32, kind="ExternalInput")
    n_d = nc.dram_tensor("noise", [N_LOC, D], f32, kind="ExternalInput")
    o_d = nc.dram_tensor("out", [N_LOC, D], f32, kind="ExternalOutput")

    def blk(t, b):
        return t[b * BLOCK_ROWS:(b + 1) * BLOCK_ROWS, :].rearrange(
            "(p q) d -> p (q d)", p=128
        )

    with tile.TileContext(nc) as tc:
        with tc.tile_pool(name="io", bufs=4) as iop, tc.tile_pool(
            name="small", bufs=4
        ) as sp:
            for b in range(N_BLOCKS):
                xt = iop.tile([128, FREE], f32, tag="x")
                nt = iop.tile([128, FREE], f32, tag="n")
                ot = iop.tile([128, FREE], f32, tag="o")
                sq = iop.tile([128, FREE], f32, tag="sq")
                ss = sp.tile([128, RPP], f32, tag="ss")
                sc = sp.tile([128, RPP], f32, tag="sc")

                nc.sync.dma_start(xt[:], blk(x_d, b))
                nc.sync.dma_start(nt[:], blk(n_d, b))

                for j in range(RPP):
                    nc.scalar.activation(
                        sq[:, j * D:(j + 1) * D],
                        xt[:, j * D:(j + 1) * D],
                        mybir.ActivationFunctionType.Square,
                        accum_out=ss[:, j:j + 1],
                    )
                nc.scalar.sqrt(ss[:], ss[:])
                nc.vector.tensor_scalar_max(ss[:], ss[:], 1.0)
                nc.vector.reciprocal(sc[:], ss[:])
                for j in range(RPP):
                    nc.vector.scalar_tensor_tensor(
                        ot[:, j * D:(j + 1) * D],
                        xt[:, j * D:(j + 1) * D],
                        sc[:, j:j + 1],
                        nt[:, j * D:(j + 1) * D],
                        op0=mybir.AluOpType.mult,
                        op1=mybir.AluOpType.add,
                    )
                nc.sync.dma_start(blk(o_d, b), ot[:])

    nc.compile()
    _NC_CACHE = nc
    return nc


def _run(x, noise, trace=False):
    from concourse.bass_utils import run_bass_kernel_spmd

    nc = _build()
    x = np.ascontiguousarray(x, dtype=np.float32)
    noise = np.ascontiguousarray(noise, dtype=np.float32)
    in_maps = [
        {
            "x": x[i * N_LOC:(i + 1) * N_LOC],
            "noise": noise[i * N_LOC:(i + 1) * N_LOC],
        }
        for i in range(NCORES)
    ]
    res = run_bass_kernel_spmd(nc, in_maps, list(range(NCORES)), trace=trace)
    out = np.concatenate([res.results[i]["out"] for i in range(NCORES)], axis=0)
    return out, res


def kernel(x, noise):
    out, _ = _run(x, noise)
    return out
